# revision 7
# baseline (speedup 1.0000x reference)
"""Cross-attention (B=2, N=M=2048, DIM=1024, H=16) on 8 TRN2 NeuronCores, v4.

Sharding: tensor-parallel over heads (2 heads/core). Key structure vs v3:
  - score matmuls for the two heads write the two banks of ONE [128,2,NB]
    psum pair tile, issued adjacently as 64-row PE tiles ((0,0)/(64,0)) so
    they stream concurrently (~512 cyc/pair instead of ~2x700).
  - ONE exp instruction per pair ([128,1024] across both banks), engines
    alternating per mt: ScalarE exact LUT (with the Schraudolph multiply
    folded into Wq host-side, scale=ln2/128 restores e^x), VectorE 1-ALU-op
    Schraudolph add -> int16 bitcast bf16.
  - 2-mt beats group same-PE-mode runs ([AV,AV,fillers][S,S]) to halve
    tiling-mode-switch drains.
  - AV keeps the D+1 ones-column (M=65): den via any other route costs
    ~95us of DVE/GpSimd adds (measured rates), worse than the half-array
    matmul waste.
  - D2 out-projection runs as fillers inside qb(1,3) (v3 left it after
    coll3 -> 17.7us tail PE gap); output stored/DMA'd as bf16.
  - chunked AllToAll: 4 collectives over 1024-token groups, overlapped with
    remaining attention compute.

Compute dtype: bf16 matmul operands, f32 PSUM accumulation, bf16 output
(f32-ified host-side).
"""

import sys

for _p in ("/opt/trn_rl_repo",):
    if _p not in sys.path:
        sys.path.append(_p)

import math

import ml_dtypes
import numpy as np

import concourse.bass as bass
import concourse.mybir as mybir
import concourse.tile as tile
from concourse import bacc

NCORES = 8
B, N, M, DIM, H = 2, 2048, 2048, 1024, 16
D = DIM // H                  # 64 head dim
HPC = H // NCORES             # 2 heads per core
DLOC = HPC * D                # 128 local q/k/v dims per core
TOK = B * N                   # 4096 query tokens
NB = 512                      # token chunk / psum bank width (f32)
KT = DIM // 128               # 8 contraction tiles for projections
MT = M // 128                 # 16 m-tiles per batch
NQB = N // NB                 # 4 query blocks per batch
NCH = TOK // NB               # 8 token chunks total
NCOLL = 4                     # collectives (2 chunks = 1024 tokens each)
TSL = TOK // NCORES           # 512 output tokens per core
SCALE = float(D) ** -0.5

# Schraudolph constants. The 128*log2(e) multiply is folded into Wq
# host-side, so scores arrive as s_pre = 128*log2e*(q.k*SCALE):
#   DVE:    bits16 = s_pre + C_SCH  (1 ALU op, truncating f32->i16)
#   Scalar: e^x = exp(s_pre * ln2/128) via the activation's free affine
K_SCH = 128.0 * math.log2(math.e)
C_SCH = 128.0 * (127.0 - 0.0436775) + 0.5   # +0.5 assumes truncating convert
EXP_SCALE = math.log(2.0) / 128.0

BF16 = mybir.dt.bfloat16
F32 = mybir.dt.float32
I16 = mybir.dt.int16
AF = mybir.ActivationFunctionType
ALU = mybir.AluOpType


def build():
    nc = bacc.Bacc("TRN2", target_bir_lowering=False, debug=False,
                   num_devices=NCORES)

    # host-pre-tiled inputs: x?t[ch] is one contiguous [128, KT, NB] block
    x1t = nc.declare_dram_parameter("x1t", [NCH, 128, KT, NB], BF16,
                                    isOutput=False)
    x2t = nc.declare_dram_parameter("x2t", [NCH, 128, KT, NB], BF16,
                                    isOutput=False)
    wq = nc.declare_dram_parameter("wq", [128, KT, DLOC], BF16, isOutput=False)
    wk = nc.declare_dram_parameter("wk", [128, KT, DLOC], BF16, isOutput=False)
    wv = nc.declare_dram_parameter("wv", [128, KT, DLOC], BF16, isOutput=False)
    wp = nc.declare_dram_parameter("wp", [128, KT, DIM], BF16, isOutput=False)
    bp = nc.declare_dram_parameter("bp", [1, DIM], BF16, isOutput=False)
    out = nc.declare_dram_parameter("out", [TSL, DIM], BF16, isOutput=True)

    # DRAM bounce buffers for the 4 chunked AllToAlls
    ata_in = [nc.dram_tensor(f"ata_in{j}", [NCORES, DLOC, 128], BF16)
              for j in range(NCOLL)]
    ata_out = [nc.dram_tensor(f"ata_out{j}", [NCORES, DLOC, 128], BF16)
               for j in range(NCOLL)]

    with tile.TileContext(nc) as tc:
        with (
            tc.tile_pool(name="persist", bufs=1) as pp,
            tc.tile_pool(name="xin", bufs=3) as xp,
            tc.tile_pool(name="ptb", bufs=4) as ptp,       # bf16 exp out (ACT)
            tc.tile_pool(name="pti", bufs=4) as ptip,      # int16 exp out (DVE)
            tc.tile_pool(name="norm", bufs=4) as np_,
            tc.tile_pool(name="yout", bufs=2) as yp,
            tc.tile_pool(name="ofp", bufs=1) as ofp,
        ):
            # ---- persistent SBUF tensors ----
            wq_sb = pp.tile([128, KT, DLOC], BF16, tag="wq")
            wk_sb = pp.tile([128, KT, DLOC], BF16, tag="wk")
            wv_sb = pp.tile([128, KT, DLOC], BF16, tag="wv")
            wp_sb = pp.tile([128, KT, DIM], BF16, tag="wp")
            bp_sb = pp.tile([1, DIM], BF16, tag="bp")
            bias_bc = pp.tile([128, DIM], BF16, tag="bias_bc")
            qt_b = [pp.tile([128, N], BF16, tag=f"qt{b}", name=f"qt{b}")
                    for b in range(B)]
            kt_b = [pp.tile([128, M], BF16, tag=f"kt{b}", name=f"kt{b}")
                    for b in range(B)]
            v_b = [pp.tile([128, MT, HPC, D + 1], BF16, tag=f"v{b}",
                           name=f"v{b}")
                   for b in range(B)]
            # normalized head-output, indexed [dloc, coll, dest_seg, 128tok]
            ot_sb = pp.tile([128, NCOLL, NCORES, 128], BF16, tag="ot")

            # batched weight DMAs; wk/wv first (B starts with k/v)
            nc.gpsimd.dma_start(wk_sb[:], wk[:])
            nc.gpsimd.dma_start(wv_sb[:], wv[:])
            nc.scalar.dma_start(wq_sb[:], wq[:])
            nc.scalar.dma_start(wp_sb[:], wp[:])
            nc.gpsimd.dma_start(bp_sb[:], bp[:])
            nc.gpsimd.partition_broadcast(bias_bc[:], bp_sb[0:1, :])
            for b in range(B):
                nc.vector.memset(v_b[b][:, :, :, D], 1.0)

            # x DMAs (sync queue): all of batch-0 x2 first (C(b0) needs full
            # k/v), then batch-0 x1, then batch 1.
            x2_tiles = {}
            x1_tiles = {}

            def load_x2(ch):
                t = xp.tile([128, KT, NB], BF16, tag="x2", bufs=5, name="x2t")
                nc.sync.dma_start(t[:], x2t[ch])
                x2_tiles[ch] = t

            def load_x1(ch):
                t = xp.tile([128, KT, NB], BF16, tag="x1", bufs=4, name="x1t")
                nc.sync.dma_start(t[:], x1t[ch])
                x1_tiles[ch] = t

            for ch in range(NQB):          # batch 0
                load_x2(ch)
            for ch in range(NQB):
                load_x1(ch)

            with (
                tc.tile_pool(name="ps_b", bufs=1, space="PSUM") as psb,
                tc.tile_pool(name="ps_s", bufs=2, space="PSUM") as pss,
                tc.tile_pool(name="ps_o", bufs=1, space="PSUM") as pso,
            ):
                # score-pair psum tiles double as B-phase scratch: b0's
                # k/v/q units draw [128,NB] halves from the pair pool.
                _half = {"t": None, "i": 0}

                def pair_half():
                    if _half["i"] % 2 == 0:
                        _half["t"] = pss.tile([128, 2, NB], F32, tag="spair",
                                              bufs=2, name="s_pair")
                    t = _half["t"]
                    h = _half["i"] % 2
                    _half["i"] += 1
                    return t[:, h, :]

                # ---------- phase B pieces ----------
                # pre-C units (batch 0) evacuate psum via the idle ScalarE
                # and draw psum from the score-pair pool; in-C filler units
                # (batch 1) use VectorE + the 1-bank B pool.
                def emit_k_unit(b, i, alloc, eng):
                    k_ps = alloc()
                    x2_t = x2_tiles[NQB * b + i]
                    for k in range(KT):
                        nc.tensor.matmul(k_ps, wk_sb[:, k, :],
                                         x2_t[:, k, :],
                                         start=(k == 0), stop=(k == KT - 1))
                    _copy(eng, kt_b[b][:, NB * i:NB * (i + 1)], k_ps)

                def emit_v_units(b, i, alloc, eng):
                    state = {}

                    def vj(j):
                        if j == 0:
                            state["v_ps"] = alloc()
                        v_ps = state["v_ps"]
                        x2_t = x2_tiles[NQB * b + i]
                        for k in range(KT):
                            nc.tensor.matmul(
                                v_ps[:, 128 * j:128 * j + DLOC],
                                x2_t[:, k, 128 * j:128 * (j + 1)],
                                wv_sb[:, k, :],
                                start=(k == 0), stop=(k == KT - 1))
                        if j == 3:
                            for hh in range(HPC):
                                _copy(eng,
                                      v_b[b][:, 4 * i:4 * i + 4, hh, 0:D],
                                      v_ps.rearrange(
                                          "p (j d) -> p j d", j=4)[
                                          :, :, D * hh:D * (hh + 1)])
                    return [lambda j=j: vj(j) for j in range(4)]

                def emit_q_unit(b, i, alloc, eng):
                    q_ps = alloc()
                    x1_t = x1_tiles[NQB * b + i]
                    for k in range(KT):
                        nc.tensor.matmul(q_ps, wq_sb[:, k, :],
                                         x1_t[:, k, :],
                                         start=(k == 0), stop=(k == KT - 1))
                    _copy(eng, qt_b[b][:, NB * i:NB * (i + 1)], q_ps)

                def b_alloc():
                    t = psb.tile([128, NB], F32, tag="bps", name="b_ps")
                    return t[:]

                def _copy(eng, dst, src):
                    if eng is nc.scalar:
                        nc.scalar.copy(dst, src)
                    else:
                        nc.vector.tensor_copy(dst, src)

                # ---------- phase D pieces ----------
                of_tiles = {}

                def emit_of_load(j):
                    of = ofp.tile([128, NCORES, 128], BF16, tag=f"of{j}",
                                  name=f"of{j}")
                    nc.sync.dma_start(
                        of[:], ata_out[j][:].rearrange("s p t -> p s t"))
                    of_tiles[j] = of

                def emit_d_units(j):
                    units = []
                    box = {}

                    def mk(eb, k):
                        def mm():
                            if eb == 0 and k == 0:
                                box["y"] = yp.tile([128, DIM], BF16,
                                                   tag="ysb", name="y_sb")
                            if k == 0:
                                box[f"ps{eb}"] = psb.tile(
                                    [128, NB], F32, tag="bps", name="y_ps")
                            y_ps = box[f"ps{eb}"]
                            esl = slice(NB * eb, NB * (eb + 1))
                            nc.tensor.matmul(y_ps[:], of_tiles[j][:, k, :],
                                             wp_sb[:, k, esl],
                                             start=(k == 0),
                                             stop=(k == NCORES - 1))
                            if k == NCORES - 1:
                                nc.vector.tensor_add(
                                    box["y"][:, esl], y_ps[:],
                                    bias_bc[:, esl])
                        return mm

                    for eb in range(DIM // NB):
                        for k in range(NCORES):
                            units.append(mk(eb, k))

                    def yout():
                        nc.sync.dma_start(out[128 * j:128 * (j + 1), :],
                                          box["y"][:])
                    units.append(yout)
                    return units

                # ---------- phase C ----------
                def emit_c_qb(b, qb, fillers):
                    ch = NQB * b + qb
                    lnsl = slice(NB * qb, NB * (qb + 1))
                    o_ps = [pso.tile([D + 1, NB], F32, tag=f"ops{hh}",
                                     bufs=(2 if hh == 0 else 1),
                                     name=f"o_ps{hh}")
                            for hh in range(HPC)]
                    pts = []

                    def scores(mt):
                        msl = slice(128 * mt, 128 * (mt + 1))
                        sp = pss.tile([128, 2, NB], F32, tag="spair",
                                      bufs=2, name="s_pair")
                        for hh in range(HPC):
                            hsl = slice(D * hh, D * (hh + 1))
                            nc.tensor.matmul(sp[:, hh, :], kt_b[b][hsl, msl],
                                             qt_b[b][hsl, lnsl],
                                             start=True, stop=True)
                        if mt % 8 in (1, 4, 6):   # DVE pair (3/8)
                            pt = ptip.tile([128, 2, NB], I16, tag="pti",
                                           name="pt_i")
                            nc.vector.tensor_scalar(
                                pt[:].rearrange("p a t -> p (a t)"),
                                sp[:].rearrange("p a t -> p (a t)"),
                                C_SCH, None, ALU.add)
                            pts.append([pt[:, hh, :].bitcast(BF16)
                                        for hh in range(HPC)])
                        else:                  # ScalarE pair
                            pt = ptp.tile([128, 2, NB], BF16, tag="ptb",
                                          name="pt_b")
                            nc.scalar.activation(
                                pt[:].rearrange("p a t -> p (a t)"),
                                sp[:].rearrange("p a t -> p (a t)"),
                                AF.Exp, scale=EXP_SCALE)
                            pts.append([pt[:, hh, :] for hh in range(HPC)])

                    def av(mt):
                        for hh in range(HPC):
                            nc.tensor.matmul(
                                o_ps[hh][:], v_b[b][:, mt, hh, :],
                                pts[mt][hh],
                                start=(mt == 0), stop=(mt == MT - 1))

                    def pop_filler():
                        if fillers:
                            fillers.pop(0)()

                    # 2-mt beats: [AV(2i-2) AV(2i-1) fillers][S(2i) S(2i+1)]
                    scores(0)
                    scores(1)
                    for i in range(1, MT // 2):
                        av(2 * i - 2)
                        pop_filler()
                        av(2 * i - 1)
                        pop_filler()
                        pop_filler()
                        scores(2 * i)
                        scores(2 * i + 1)
                    av(MT - 2)
                    av(MT - 1)
                    while fillers:
                        fillers.pop(0)()
                    # normalize: 1/den (fast approx), broadcast, multiply.
                    # h1 first: o_ps1 is single-buffered, clear it earliest.
                    j, par = ch // 2, ch % 2
                    for hh in (1, 0):
                        hsl = slice(D * hh, D * (hh + 1))
                        dn = np_.tile([1, NB], F32, tag="den", name="dn")
                        nc.vector.tensor_copy(dn[:], o_ps[hh][D:D + 1, :])
                        rc = np_.tile([1, NB], F32, tag="recip", name="rc")
                        nc.vector.reciprocal_approx_fast(rc[:], dn[:])
                        bc = np_.tile([D, NB], F32, tag="bcast", name="bc")
                        nc.gpsimd.partition_broadcast(bc[:], rc[0:1, :])
                        nc.vector.tensor_mul(
                            ot_sb[hsl, j, 4 * par:4 * par + 4, :].rearrange(
                                "p a t -> p (a t)"),
                            o_ps[hh][0:D, :], bc[:])

                def emit_coll(j):
                    # gpsimd queue: lands right before its own trigger and
                    # bypasses the sync FIFO (x loads would delay it there)
                    nc.gpsimd.dma_start(
                        ata_in[j][:].rearrange("s p t -> p s t"),
                        ot_sb[:, j, :, :])
                    nc.gpsimd.collective_compute(
                        "AllToAll", mybir.AluOpType.bypass,
                        replica_groups=[list(range(NCORES))],
                        ins=[ata_in[j].ap().opt()],
                        outs=[ata_out[j].ap().opt()],
                    )
                    emit_of_load(j)

                # ---------- schedule ----------
                for i in range(NQB):
                    emit_k_unit(0, i, pair_half, nc.scalar)
                    for u in emit_v_units(0, i, pair_half, nc.scalar):
                        u()
                emit_q_unit(0, 0, pair_half, nc.scalar)
                for ch in range(NQB, NCH):
                    load_x2(ch)
                for ch in range(NQB, NCH):
                    load_x1(ch)

                def b1_units(i):
                    return ([lambda i=i: emit_k_unit(1, i, b_alloc,
                                                     nc.vector)]
                            + emit_v_units(1, i, b_alloc, nc.vector)
                            + [lambda i=i: emit_q_unit(1, i, b_alloc,
                                                       nc.vector)])

                emit_c_qb(0, 0, [
                    lambda: emit_q_unit(0, 1, b_alloc, nc.vector),
                    lambda: emit_q_unit(0, 2, b_alloc, nc.vector),
                    lambda: emit_q_unit(0, 3, b_alloc, nc.vector)])
                emit_c_qb(0, 1, b1_units(0) + b1_units(1))
                emit_coll(0)
                emit_c_qb(0, 2, b1_units(2) + b1_units(3))
                d0 = emit_d_units(0)
                emit_c_qb(0, 3, d0[:21])
                emit_coll(1)
                emit_c_qb(1, 0, d0[21:])
                d1 = emit_d_units(1)
                emit_c_qb(1, 1, d1[:21])
                emit_coll(2)
                d2 = emit_d_units(2)
                emit_c_qb(1, 2, d1[21:] + d2[:9])
                emit_c_qb(1, 3, d2[9:])
                emit_coll(3)
                for u in emit_d_units(3):
                    u()

    nc.compile()
    return nc


def _tile_xt(x):
    """[B,N,DIM] f32 -> [TOK//NB, 128, KT, NB] bf16 block-contiguous x^T."""
    bf = ml_dtypes.bfloat16
    xt = x.reshape(TOK, DIM).T
    return np.ascontiguousarray(
        xt.reshape(KT, 128, TOK // NB, NB).transpose(2, 1, 0, 3)).astype(bf)


def make_in_maps(x1, x2, Wq, Wkv, Wproj, bproj):
    bf = ml_dtypes.bfloat16
    x1t = _tile_xt(x1)
    x2t = _tile_xt(x2)
    wq_full = Wq * (SCALE * K_SCH)   # scores arrive pre-scaled for exp
    wk_full = Wkv[:, :DIM]
    wv_full = Wkv[:, DIM:]
    wp = np.ascontiguousarray(
        Wproj.reshape(KT, 128, DIM).transpose(1, 0, 2)).astype(bf)
    bp = bproj.reshape(1, DIM).astype(bf)
    in_maps = []
    for c in range(NCORES):
        sl = slice(DLOC * c, DLOC * (c + 1))
        in_maps.append({
            "x1t": x1t, "x2t": x2t,
            "wq": np.ascontiguousarray(
                wq_full[:, sl].reshape(KT, 128, DLOC).transpose(1, 0, 2)).astype(bf),
            "wk": np.ascontiguousarray(
                wk_full[:, sl].reshape(KT, 128, DLOC).transpose(1, 0, 2)).astype(bf),
            "wv": np.ascontiguousarray(
                wv_full[:, sl].reshape(KT, 128, DLOC).transpose(1, 0, 2)).astype(bf),
            "wp": wp, "bp": bp,
        })
    return in_maps


_nc = None


def run(inputs, trace=False):
    """Returns (full_output [B,N,DIM] f32, BassKernelResults)."""
    global _nc
    from concourse.bass_utils import run_bass_kernel_spmd
    if _nc is None:
        _nc = build()
    in_maps = make_in_maps(**inputs)
    res = run_bass_kernel_spmd(_nc, in_maps, core_ids=list(range(NCORES)),
                               trace=trace)
    # unshard: core c's out rows [128j : 128j+128] are global tokens
    # 512*(2j + c//4) + 128*(c%4) .. +128
    y = np.empty((TOK, DIM), dtype=np.float32)
    for c in range(NCORES):
        oc = res.results[c]["out"]
        for j in range(NCOLL):
            g = 512 * (2 * j + c // 4) + 128 * (c % 4)
            y[g:g + 128] = oc[128 * j:128 * (j + 1)].astype(np.float32)
    return y.reshape(B, N, DIM), res


def kernel(x1, x2, Wq, Wkv, Wproj, bproj):
    y, _ = run(dict(x1=x1, x2=x2, Wq=Wq, Wkv=Wkv, Wproj=Wproj, bproj=bproj))
    return y
